# revision 1
# baseline (speedup 1.0000x reference)
"""Trainium2 Bass kernel for nn_AudioModel segment_reduce.

Reference computation (per batch row b):
  - frames t < audio_lengths[b] are valid
  - consecutive runs of equal phoneme_ids form segments
  - feat[b] = mean over segments of (mean over frames in segment of h[b,t,:])
  - logit[b] = feat[b] @ W.T + bias

Algebraic collapse: feat[b] = sum_t w[b,t] * h[b,t,:] with
  w[b,t] = valid[b,t] / (run_len(b, run_of(t)) * n_runs[b])
so  logit[b] = sum_t w[b,t] * (h[b,t,:] . W) + bias.

The per-frame weights w depend only on the tiny phoneme_ids/audio_lengths
tensors and are computed on host. The 588MB hidden_states tensor is streamed
through the device exactly once (memory-bound): per batch row, the T
contraction runs on DVE as a chain of fused scalar_tensor_tensor ops
(acc += w_chunk * h_chunk, per-partition scalar = per-frame weight), the
remaining chunks contract on the PE as fp32 matmuls accumulating in PSUM
(the DVE accumulator folds into the same PSUM group via a ones-matmul), and
one fused DVE op applies the classifier dot. Full fp32 throughout. Pure data
parallel: 16 batch rows per NeuronCore; each row's loads are split into
~1.1MB 3-chunk pieces across the two HWDGE rings (sync/scalar feed DVE/PE
respectively, alternating per row) with the ragged 88-frame tail on SWDGE,
so the HBM stream stays dense end-to-end and both pipeline edges are short.
"""

import numpy as np

B, T, C = 128, 1496, 768
NCORES = 8
RPC = B // NCORES          # batch rows per core
NCHUNK = 12                # t-chunks of 128 frames
NTAIL = T - 11 * 128       # 88 frames in the last chunk
DVE_CHUNKS = 6             # chunks 0..5 + tail on DVE, 6..10 on PE

_CACHE = {}


def _frame_weights(phoneme_ids, audio_lengths):
    """w[b,t] = valid / (run_length(run containing t) * n_runs[b]); 0 if invalid."""
    pid = np.asarray(phoneme_ids)
    L = np.asarray(audio_lengths).astype(np.int64)
    t = np.arange(T)
    valid = t[None, :] < L[:, None]                               # (B, T)
    change = pid[:, 1:] != pid[:, :-1]
    boundary = np.concatenate([np.ones((B, 1), bool), change], axis=1) & valid
    seg = np.cumsum(boundary, axis=1) - 1
    np.maximum(seg, 0, out=seg)                                   # (B, T)
    gid = (seg + np.arange(B, dtype=np.int64)[:, None] * T).ravel()
    cnt = np.bincount(gid, weights=valid.ravel().astype(np.float64), minlength=B * T)
    cnt_t = cnt[gid].reshape(B, T)                                # run length per frame
    n_runs = boundary.sum(axis=1).astype(np.float64)              # (B,)
    w = np.where(valid, 1.0 / (np.maximum(cnt_t, 1.0) * n_runs[:, None]), 0.0)
    return w.astype(np.float32)


def _weight_cols(w):
    """(B, 128, NCHUNK): [:, p, ci] = weight of frame ci*128+p (zeros pad the tail)."""
    wc = np.zeros((B, 128, NCHUNK), dtype=np.float32)
    wc[:, :, :11] = w[:, : 11 * 128].reshape(B, 11, 128).transpose(0, 2, 1)
    wc[:, :NTAIL, 11] = w[:, 11 * 128 :]
    return wc


def _build_program():
    import concourse.bacc as bacc
    import concourse.tile as tile
    from concourse import mybir

    f32 = mybir.dt.float32

    nc = bacc.Bacc("TRN2", target_bir_lowering=False, debug=False)
    h = nc.dram_tensor("h", [RPC, T, C], f32, kind="ExternalInput").ap()
    wt = nc.dram_tensor("wt", [128, RPC * NCHUNK], f32, kind="ExternalInput").ap()
    wv = nc.dram_tensor("wv", [1, C], f32, kind="ExternalInput").ap()
    out = nc.dram_tensor("out", [RPC, 1], f32, kind="ExternalOutput").ap()

    with tile.TileContext(nc) as tc:
        with (
            tc.tile_pool(name="hp", bufs=4) as hp,
            tc.tile_pool(name="const", bufs=1) as cp,
            tc.tile_pool(name="ap_", bufs=4) as apool,
            tc.tile_pool(name="ps", bufs=4, space="PSUM") as pp,
            tc.tile_pool(name="fin", bufs=3) as fp,
        ):
            wtile = cp.tile([128, RPC * NCHUNK], f32)
            nc.sync.dma_start(wtile[:], wt)
            wvt = cp.tile([1, C], f32)
            nc.scalar.dma_start(wvt[:], wv)
            logits = cp.tile([1, RPC], f32)
            ones = cp.tile([128, 1], f32)
            nc.vector.memset(ones[:], 1.0)

            NA = DVE_CHUNKS            # chunks 0..NA-1 -> DVE (tile A, ring 0)
            NB = 11 - NA               # chunks NA..10  -> PE  (tile B, ring 1)
            for r in range(RPC):
                # Per-row loads are split so each engine's data arrives on its
                # own ring and compute starts after a half-row, not a full row:
                #   A: chunks 0..NA-1  -> DVE accumulator chain
                #   B: chunks NA..10   -> PE fp32 matmul group
                #   T: ragged 88-frame tail (SWDGE) -> one more DVE chain op
                ra, rb = (nc.sync, nc.scalar) if r % 2 == 0 else (nc.scalar, nc.sync)
                hA = hp.tile([128, NA * C], f32, tag="hA")
                hA3 = hA.rearrange("p (ci c) -> p ci c", c=C)
                for lo, hi in ((0, 3), (3, NA)):
                    ra.dma_start(
                        hA3[:, lo:hi, :],
                        h[r, lo * 128 : hi * 128, :].rearrange(
                            "(ci p) c -> p ci c", p=128
                        ),
                    )
                hB = hp.tile([128, NB * C], f32, tag="hB")
                hB3 = hB.rearrange("p (ci c) -> p ci c", c=C)
                for lo, hi in ((0, 3), (3, NB)):
                    rb.dma_start(
                        hB3[:, lo:hi, :],
                        h[r, (NA + lo) * 128 : (NA + hi) * 128, :].rearrange(
                            "(ci p) c -> p ci c", p=128
                        ),
                    )
                hT = hp.tile([128, C], f32, tag="hT")
                ra.dma_start(hT[:NTAIL, :], h[r, 11 * 128 :, :])

                col = lambda ci: wtile[:, r * NCHUNK + ci : r * NCHUNK + ci + 1]
                acc = apool.tile([128, C], f32, tag="acc")
                nc.vector.tensor_scalar_mul(acc[:], hA[:, :C], col(0))
                for ci in range(1, NA):
                    nc.vector.scalar_tensor_tensor(
                        out=acc[:],
                        in0=hA[:, ci * C : (ci + 1) * C],
                        scalar=col(ci),
                        in1=acc[:],
                        op0=mybir.AluOpType.mult,
                        op1=mybir.AluOpType.add,
                    )
                nc.vector.scalar_tensor_tensor(
                    out=acc[:NTAIL],
                    in0=hT[:NTAIL, :],
                    scalar=col(11)[:NTAIL],
                    in1=acc[:NTAIL],
                    op0=mybir.AluOpType.mult,
                    op1=mybir.AluOpType.add,
                )

                p = pp.tile([1, C], f32, tag="p")
                for k, ci in enumerate(range(NA, 11)):
                    lw = col(ci)
                    nc.tensor.matmul(
                        p[:, :512], lw, hB3[:, k, :512], start=(k == 0), stop=False
                    )
                    nc.tensor.matmul(
                        p[:, 512:], lw, hB3[:, k, 512:], start=(k == 0), stop=False
                    )
                nc.tensor.matmul(p[:, :512], ones[:], acc[:, :512], start=False, stop=True)
                nc.tensor.matmul(p[:, 512:], ones[:], acc[:, 512:], start=False, stop=True)
                sc = fp.tile([1, C], f32, tag="sc")
                nc.vector.scalar_tensor_tensor(
                    out=sc[:],
                    in0=p[:],
                    scalar=1.0,
                    in1=wvt[:],
                    op0=mybir.AluOpType.mult,
                    op1=mybir.AluOpType.mult,
                    accum_out=logits[:, r : r + 1],
                )

            nc.sync.dma_start(out.rearrange("r o -> o r"), logits[:])

    nc.compile()
    return nc


def _get_program():
    if "nc" not in _CACHE:
        _CACHE["nc"] = _build_program()
    return _CACHE["nc"]


def _run(inputs, trace=False):
    from concourse.bass_utils import run_bass_kernel_spmd

    hidden = np.ascontiguousarray(np.asarray(inputs["hidden_states"], dtype=np.float32))
    W = np.ascontiguousarray(np.asarray(inputs["W"], dtype=np.float32))
    bias = np.asarray(inputs["b"], dtype=np.float32)
    w = _frame_weights(inputs["phoneme_ids"], inputs["audio_lengths"])
    wc = _weight_cols(w)                                          # (B, 128, NCHUNK)

    in_maps = []
    for i in range(NCORES):
        r0 = i * RPC
        wt_core = np.ascontiguousarray(
            wc[r0 : r0 + RPC].transpose(1, 0, 2).reshape(128, RPC * NCHUNK)
        )
        in_maps.append(
            {
                "h": hidden[r0 : r0 + RPC],
                "wt": wt_core,
                "wv": W.reshape(1, C),
            }
        )

    nc = _get_program()
    res = run_bass_kernel_spmd(nc, in_maps, list(range(NCORES)), trace=trace)
    logit = np.concatenate([res.results[i]["out"] for i in range(NCORES)], axis=0)
    logit = logit + bias[None, :]
    return logit.astype(np.float32), res


def kernel(**inputs):
    return _run(inputs, trace=False)[0]



# revision 2
# speedup vs baseline: 2.1025x; 2.1025x over previous
"""Trainium2 Bass kernel for nn_AudioModel segment_reduce.

Reference computation (per batch row b):
  - frames t < audio_lengths[b] are valid
  - consecutive runs of equal phoneme_ids form segments
  - feat[b] = mean over segments of (mean over frames in segment of h[b,t,:])
  - logit[b] = feat[b] @ W.T + bias

Algebraic collapse: feat[b] = sum_t w[b,t] * h[b,t,:] with
  w[b,t] = valid[b,t] / (run_len(b, run_of(t)) * n_runs[b])
so  logit[b] = sum_t w[b,t] * (h[b,t,:] . W) + bias.

The per-frame weights w depend only on the tiny phoneme_ids/audio_lengths
tensors and are computed on host. hidden_states is the only heavy tensor;
the kernel is memory-bound on streaming it, so the device stream is
minimized two ways:
  - fp16 storage (host casts; PE multiplies fp16 h by fp16 per-frame
    weights, accumulating fp32 in PSUM) — halves traffic, max rel err
    ~4e-3 vs the 2e-2 gate;
  - frames past audio_length have zero weight and are never shipped:
    each row is packed to ceil(len/128) 128-frame chunks. Rows are
    dealt to the 8 cores by descending chunk count (rank k goes to slot
    k//8), so all cores run one identical program whose per-slot chunk
    counts are the max over that rank octet (~2% padding).
Per slot: one big HWDGE DMA (rings alternate per slot) loads the packed
[128, nch*C] fp16 tile (partition-major in DRAM, so descriptors are 128
long contiguous lines); the T-contraction runs on the PE as a chain of
(128,1)x(128,512|256) fp16 matmuls accumulating into a (1,C) fp32 PSUM
group; one fused DVE op applies the classifier dot straight out of PSUM.
"""

import numpy as np

B, T, C = 128, 1496, 768
NCORES = 8
RPC = B // NCORES          # batch rows (slots) per core
PF = 128                   # frames per chunk

_CACHE = {}


def _frame_weights(phoneme_ids, audio_lengths):
    """w[b,t] = valid / (run_length(run containing t) * n_runs[b]); 0 if invalid."""
    pid = np.asarray(phoneme_ids)
    L = np.asarray(audio_lengths).astype(np.int64)
    t = np.arange(T)
    valid = t[None, :] < L[:, None]                               # (B, T)
    change = pid[:, 1:] != pid[:, :-1]
    boundary = np.concatenate([np.ones((B, 1), bool), change], axis=1) & valid
    seg = np.cumsum(boundary, axis=1) - 1
    np.maximum(seg, 0, out=seg)                                   # (B, T)
    gid = (seg + np.arange(B, dtype=np.int64)[:, None] * T).ravel()
    cnt = np.bincount(gid, weights=valid.ravel().astype(np.float64), minlength=B * T)
    cnt_t = cnt[gid].reshape(B, T)                                # run length per frame
    n_runs = boundary.sum(axis=1).astype(np.float64)              # (B,)
    w = np.where(valid, 1.0 / (np.maximum(cnt_t, 1.0) * n_runs[:, None]), 0.0)
    return w.astype(np.float32)


def _plan(audio_lengths):
    """Deal rows to cores by descending chunk count; slot k takes ranks 8k..8k+7."""
    L = np.minimum(np.asarray(audio_lengths).astype(np.int64), T)
    nch = np.ceil(L / PF).astype(np.int64)                        # chunks per row
    order = np.argsort(-nch, kind="stable")                       # rank -> row
    # rows[i][k] = row handled by core i in slot k
    rows = [[int(order[8 * k + i]) for k in range(RPC)] for i in range(NCORES)]
    NCH = tuple(int(nch[order[8 * k]]) for k in range(RPC))       # per-slot chunk count
    return rows, nch, NCH


def _build_program(NCH):
    import concourse.bacc as bacc
    import concourse.tile as tile
    from concourse import mybir

    f16 = mybir.dt.float16
    f32 = mybir.dt.float32
    TOTCH = sum(NCH)

    nc = bacc.Bacc("TRN2", target_bir_lowering=False, debug=False)
    h = nc.dram_tensor("h", [128, TOTCH * C], f16, kind="ExternalInput").ap()
    wt = nc.dram_tensor("wt", [128, TOTCH], f16, kind="ExternalInput").ap()
    wv = nc.dram_tensor("wv", [1, C], f32, kind="ExternalInput").ap()
    out = nc.dram_tensor("out", [RPC, 1], f32, kind="ExternalOutput").ap()

    with tile.TileContext(nc) as tc:
        with (
            tc.tile_pool(name="hp", bufs=5) as hp,
            tc.tile_pool(name="const", bufs=1) as cp,
            tc.tile_pool(name="ps", bufs=4, space="PSUM") as pp,
            tc.tile_pool(name="fin", bufs=3) as fp,
        ):
            wtile = cp.tile([128, TOTCH], f16)
            nc.scalar.dma_start(wtile[:], wt)
            wvt = cp.tile([1, C], f32)
            nc.scalar.dma_start(wvt[:], wv)
            logits = cp.tile([1, RPC], f32)

            off = 0
            for k, n in enumerate(NCH):
                ring = nc.sync if k % 2 == 0 else nc.scalar
                ht = hp.tile([128, n * C], f16, tag="h")
                ring.dma_start(ht[:], h[:, off * C : (off + n) * C])

                p = pp.tile([1, C], f32, tag="p")
                for j in range(n):
                    lw = wtile[:, off + j : off + j + 1]
                    nc.tensor.matmul(
                        p[:, :512], lw, ht[:, j * C : j * C + 512],
                        start=(j == 0), stop=(j == n - 1),
                    )
                    nc.tensor.matmul(
                        p[:, 512:], lw, ht[:, j * C + 512 : (j + 1) * C],
                        start=(j == 0), stop=(j == n - 1),
                    )
                sc = fp.tile([1, C], f32, tag="sc")
                nc.vector.scalar_tensor_tensor(
                    out=sc[:],
                    in0=p[:],
                    scalar=1.0,
                    in1=wvt[:],
                    op0=mybir.AluOpType.mult,
                    op1=mybir.AluOpType.mult,
                    accum_out=logits[:, k : k + 1],
                )
                off += n

            nc.sync.dma_start(out.rearrange("r o -> o r"), logits[:])

    nc.compile()
    return nc


def _get_program(NCH):
    if NCH not in _CACHE:
        _CACHE[NCH] = _build_program(NCH)
    return _CACHE[NCH]


def _pack(hidden, w, rows, nch, NCH):
    """Per-core packed fp16 frame data (partition-major) and weight columns."""
    TOTCH = sum(NCH)
    offs = np.concatenate([[0], np.cumsum(NCH)]).astype(int)
    h_maps, wt_maps = [], []
    for i in range(NCORES):
        hbuf = np.zeros((128, TOTCH, C), dtype=np.float16)
        wbuf = np.zeros((128, TOTCH), dtype=np.float16)
        for k in range(RPC):
            b = rows[i][k]
            n = int(nch[b])
            m = min(n * PF, T)
            off = offs[k]
            hrow = np.zeros((n * PF, C), dtype=np.float16)
            hrow[:m] = hidden[b, :m]
            hbuf[:, off : off + n, :] = hrow.reshape(n, PF, C).transpose(1, 0, 2)
            wrow = np.zeros(n * PF, dtype=np.float16)
            wrow[:m] = w[b, :m]
            wbuf[:, off : off + n] = wrow.reshape(n, PF).T
        h_maps.append(hbuf.reshape(128, TOTCH * C))
        wt_maps.append(wbuf)
    return h_maps, wt_maps


def _run(inputs, trace=False):
    from concourse.bass_utils import run_bass_kernel_spmd

    hidden = np.asarray(inputs["hidden_states"], dtype=np.float32)
    W = np.ascontiguousarray(np.asarray(inputs["W"], dtype=np.float32)).reshape(1, C)
    bias = np.asarray(inputs["b"], dtype=np.float32)
    w = _frame_weights(inputs["phoneme_ids"], inputs["audio_lengths"])
    rows, nch, NCH = _plan(inputs["audio_lengths"])
    h_maps, wt_maps = _pack(hidden, w, rows, nch, NCH)

    in_maps = [
        {"h": h_maps[i], "wt": wt_maps[i], "wv": W} for i in range(NCORES)
    ]

    nc = _get_program(NCH)
    res = run_bass_kernel_spmd(nc, in_maps, list(range(NCORES)), trace=trace)
    logit = np.empty((B, 1), dtype=np.float32)
    for i in range(NCORES):
        o = res.results[i]["out"]
        for k in range(RPC):
            logit[rows[i][k]] = o[k]
    logit = logit + bias[None, :]
    return logit.astype(np.float32), res


def kernel(**inputs):
    return _run(inputs, trace=False)[0]


# revision 3
# speedup vs baseline: 2.3266x; 1.1066x over previous
"""Trainium2 Bass kernel for nn_AudioModel segment_reduce.

Reference computation (per batch row b):
  - frames t < audio_lengths[b] are valid
  - consecutive runs of equal phoneme_ids form segments
  - feat[b] = mean over segments of (mean over frames in segment of h[b,t,:])
  - logit[b] = feat[b] @ W.T + bias

Algebraic collapse: feat[b] = sum_t w[b,t] * h[b,t,:] with
  w[b,t] = valid[b,t] / (run_len(b, run_of(t)) * n_runs[b])
so  logit[b] = sum_t w[b,t] * (h[b,t,:] . W) + bias.

The per-frame weights w depend only on the tiny phoneme_ids/audio_lengths
tensors and are computed on host. hidden_states is the only heavy tensor
and the kernel is memory-bound on streaming it, so the device stream is
minimized:
  - fp16 storage (host casts; PE multiplies fp16 h by fp16 per-frame
    weights, accumulating fp32 in PSUM) — halves traffic; max rel err
    ~4e-3 vs the 2e-2 gate (bf16/fp8 fail it);
  - frames past audio_length have zero weight and are never shipped: ALL
    valid frames are concatenated into one global stream that is cut into
    8 equal per-core streams (a row split by a cut contributes partial
    sums that the host adds). Every core therefore moves exactly the same
    minimal byte count and runs one identical program.
Which batch row each frame belongs to is data, not program structure:
each 128-frame chunk's matmul uses an M=SEGS stationary weight matrix
whose column s holds the frame weights of that core's segment s (zeros
elsewhere), accumulating into one long-lived (SEGS, C) fp32 PSUM group.
One fused DVE op then applies the classifier dot for all segments at
once. h arrives partition-major via ~0.8MB HWDGE DMAs alternating the
two rings (4 chunks each, so PE trails the stream closely); the weight
matrix rides the SWDGE ring so the HWDGE rings carry only h.
"""

import numpy as np

B, T, C = 128, 1496, 768
NCORES = 8
PF = 128                   # frames per chunk
CPT = 4                    # chunks per DMA piece

_CACHE = {}


def _frame_weights(phoneme_ids, audio_lengths):
    """w[b,t] = valid / (run_length(run containing t) * n_runs[b]); 0 if invalid."""
    pid = np.asarray(phoneme_ids)
    L = np.asarray(audio_lengths).astype(np.int64)
    t = np.arange(T)
    valid = t[None, :] < L[:, None]                               # (B, T)
    change = pid[:, 1:] != pid[:, :-1]
    boundary = np.concatenate([np.ones((B, 1), bool), change], axis=1) & valid
    seg = np.cumsum(boundary, axis=1) - 1
    np.maximum(seg, 0, out=seg)                                   # (B, T)
    gid = (seg + np.arange(B, dtype=np.int64)[:, None] * T).ravel()
    cnt = np.bincount(gid, weights=valid.ravel().astype(np.float64), minlength=B * T)
    cnt_t = cnt[gid].reshape(B, T)                                # run length per frame
    n_runs = boundary.sum(axis=1).astype(np.float64)              # (B,)
    w = np.where(valid, 1.0 / (np.maximum(cnt_t, 1.0) * n_runs[:, None]), 0.0)
    return w.astype(np.float32)


def _plan(audio_lengths):
    """Cut the global valid-frame stream into 8 equal per-core streams.

    Returns per-core segment lists [(row, lo, hi), ...] plus the uniform
    NCHUNK (128-frame chunks per core) and SEGS (max segments per core).
    """
    L = np.minimum(np.asarray(audio_lengths).astype(np.int64), T)
    cum = np.concatenate([[0], np.cumsum(L)])
    F = int(cum[-1])
    cuts = [(F * i) // NCORES for i in range(NCORES + 1)]
    segs = []
    for i in range(NCORES):
        lo_g, hi_g = cuts[i], cuts[i + 1]
        b0 = int(np.searchsorted(cum, lo_g, side="right")) - 1
        s = []
        g = lo_g
        b = b0
        while g < hi_g:
            e = min(int(cum[b + 1]), hi_g)
            s.append((b, g - int(cum[b]), e - int(cum[b])))
            g = e
            b += 1
        segs.append(s)
    SEGS = max(len(s) for s in segs)
    FRC = max(cuts[i + 1] - cuts[i] for i in range(NCORES))
    NCHUNK = -(-FRC // PF)
    return segs, NCHUNK, SEGS


def _build_program(NCHUNK, SEGS):
    import concourse.bacc as bacc
    import concourse.tile as tile
    from concourse import mybir

    f16 = mybir.dt.float16
    f32 = mybir.dt.float32

    nc = bacc.Bacc("TRN2", target_bir_lowering=False, debug=False)
    h = nc.dram_tensor("h", [128, NCHUNK * C], f16, kind="ExternalInput").ap()
    wt = nc.dram_tensor("wt", [128, NCHUNK * SEGS], f16, kind="ExternalInput").ap()
    wv = nc.dram_tensor("wv", [SEGS, C], f32, kind="ExternalInput").ap()
    out = nc.dram_tensor("out", [SEGS, 1], f32, kind="ExternalOutput").ap()

    npieces = -(-NCHUNK // CPT)

    with tile.TileContext(nc) as tc:
        with (
            tc.tile_pool(name="hp", bufs=8) as hp,
            tc.tile_pool(name="const", bufs=1) as cp,
            tc.tile_pool(name="ps", bufs=1, space="PSUM") as pp,
            tc.tile_pool(name="fin", bufs=1) as fp,
        ):
            wtile = cp.tile([128, NCHUNK * SEGS], f16)
            nc.gpsimd.dma_start(wtile[:], wt)
            wvt = cp.tile([SEGS, C], f32)
            nc.gpsimd.dma_start(wvt[:], wv)
            logits = cp.tile([SEGS, 1], f32)

            ps = pp.tile([SEGS, C], f32)
            for pc in range(npieces):
                n = min(CPT, NCHUNK - pc * CPT)
                ring = nc.sync if pc % 2 == 0 else nc.scalar
                ht = hp.tile([128, CPT * C], f16, tag="h")
                ring.dma_start(
                    ht[:, : n * C], h[:, pc * CPT * C : (pc * CPT + n) * C]
                )
                for j in range(n):
                    c = pc * CPT + j
                    lw = wtile[:, c * SEGS : (c + 1) * SEGS]
                    nc.tensor.matmul(
                        ps[:, :512], lw, ht[:, j * C : j * C + 512],
                        start=(c == 0), stop=(c == NCHUNK - 1),
                    )
                    nc.tensor.matmul(
                        ps[:, 512:], lw, ht[:, j * C + 512 : (j + 1) * C],
                        start=(c == 0), stop=(c == NCHUNK - 1),
                    )

            sc = fp.tile([SEGS, C], f32)
            nc.vector.scalar_tensor_tensor(
                out=sc[:],
                in0=ps[:],
                scalar=1.0,
                in1=wvt[:],
                op0=mybir.AluOpType.mult,
                op1=mybir.AluOpType.mult,
                accum_out=logits[:],
            )
            nc.sync.dma_start(out, logits[:])

    nc.compile()
    return nc


def _get_program(NCHUNK, SEGS):
    key = (NCHUNK, SEGS)
    if key not in _CACHE:
        _CACHE[key] = _build_program(*key)
    return _CACHE[key]


def _pack(hidden, w, segs, NCHUNK, SEGS):
    """Per-core packed fp16 frame stream (partition-major) and weight matrix."""
    h_maps, wt_maps = [], []
    for s in segs:
        hbuf = np.zeros((NCHUNK * PF, C), dtype=np.float16)
        q = 0
        wvals = []
        sids = []
        for sid, (b, lo, hi) in enumerate(s):
            n = hi - lo
            hbuf[q : q + n] = hidden[b, lo:hi]
            wvals.append(w[b, lo:hi])
            sids.append(np.full(n, sid, dtype=np.int64))
            q += n
        h_maps.append(
            np.ascontiguousarray(
                hbuf.reshape(NCHUNK, PF, C).transpose(1, 0, 2)
            ).reshape(128, NCHUNK * C)
        )
        wvals = np.concatenate(wvals).astype(np.float16)
        sids = np.concatenate(sids)
        pos = np.arange(q)
        wbuf = np.zeros((128, NCHUNK * SEGS), dtype=np.float16)
        wbuf[pos % PF, (pos // PF) * SEGS + sids] = wvals
        wt_maps.append(wbuf)
    return h_maps, wt_maps


def _run(inputs, trace=False):
    from concourse.bass_utils import run_bass_kernel_spmd

    hidden = np.asarray(inputs["hidden_states"], dtype=np.float32)
    W = np.ascontiguousarray(np.asarray(inputs["W"], dtype=np.float32)).reshape(1, C)
    bias = np.asarray(inputs["b"], dtype=np.float32)
    w = _frame_weights(inputs["phoneme_ids"], inputs["audio_lengths"])
    segs, NCHUNK, SEGS = _plan(inputs["audio_lengths"])
    h_maps, wt_maps = _pack(hidden, w, segs, NCHUNK, SEGS)
    wv_rep = np.ascontiguousarray(np.repeat(W, SEGS, axis=0))

    in_maps = [
        {"h": h_maps[i], "wt": wt_maps[i], "wv": wv_rep} for i in range(NCORES)
    ]

    nc = _get_program(NCHUNK, SEGS)
    res = run_bass_kernel_spmd(nc, in_maps, list(range(NCORES)), trace=trace)
    logit = np.zeros((B, 1), dtype=np.float64)
    for i in range(NCORES):
        o = res.results[i]["out"]
        for sid, (b, lo, hi) in enumerate(segs[i]):
            logit[b, 0] += float(o[sid, 0])
    logit = logit.astype(np.float32) + bias[None, :]
    return logit.astype(np.float32), res


def kernel(**inputs):
    return _run(inputs, trace=False)[0]


# revision 4
# speedup vs baseline: 2.3485x; 1.0094x over previous
"""Trainium2 Bass kernel for nn_AudioModel segment_reduce.

Reference computation (per batch row b):
  - frames t < audio_lengths[b] are valid
  - consecutive runs of equal phoneme_ids form segments
  - feat[b] = mean over segments of (mean over frames in segment of h[b,t,:])
  - logit[b] = feat[b] @ W.T + bias

Algebraic collapse: feat[b] = sum_t w[b,t] * h[b,t,:] with
  w[b,t] = valid[b,t] / (run_len(b, run_of(t)) * n_runs[b])
so  logit[b] = sum_t w[b,t] * (h[b,t,:] . W) + bias.

The per-frame weights w depend only on the tiny phoneme_ids/audio_lengths
tensors and are computed on host. hidden_states is the only heavy tensor
and the kernel is memory-bound on streaming it, so the device stream is
minimized:
  - fp16 storage (host casts; PE multiplies fp16 h by fp16 per-frame
    weights, accumulating fp32 in PSUM) — halves traffic; max rel err
    ~4e-3 vs the 2e-2 gate (bf16/fp8 fail it);
  - frames past audio_length have zero weight and are never shipped: ALL
    valid frames are concatenated into one global stream that is cut into
    8 equal per-core streams (a row split by a cut contributes partial
    sums that the host adds). Every core therefore moves exactly the same
    minimal byte count and runs one identical program.
Which batch row each frame belongs to is data, not program structure:
each 128-frame chunk's matmul uses an M=SEGS stationary weight matrix
whose column s holds the frame weights of that core's segment s (zeros
elsewhere), accumulating into one long-lived (SEGS, C) fp32 PSUM group.
One fused DVE op then applies the classifier dot for all segments at
once. h arrives partition-major via ~0.8MB HWDGE DMAs alternating the
two rings (4 chunks each, so PE trails the stream closely); the weight
matrix rides the SWDGE ring so the HWDGE rings carry only h.
"""

import numpy as np

B, T, C = 128, 1496, 768
NCORES = 8
PF = 128                   # frames per chunk
CPT = 4                    # chunks per DMA piece

_CACHE = {}


def _frame_weights(phoneme_ids, audio_lengths):
    """w[b,t] = valid / (run_length(run containing t) * n_runs[b]); 0 if invalid."""
    pid = np.asarray(phoneme_ids)
    L = np.asarray(audio_lengths).astype(np.int64)
    t = np.arange(T)
    valid = t[None, :] < L[:, None]                               # (B, T)
    change = pid[:, 1:] != pid[:, :-1]
    boundary = np.concatenate([np.ones((B, 1), bool), change], axis=1) & valid
    seg = np.cumsum(boundary, axis=1) - 1
    np.maximum(seg, 0, out=seg)                                   # (B, T)
    gid = (seg + np.arange(B, dtype=np.int64)[:, None] * T).ravel()
    cnt = np.bincount(gid, weights=valid.ravel().astype(np.float64), minlength=B * T)
    cnt_t = cnt[gid].reshape(B, T)                                # run length per frame
    n_runs = boundary.sum(axis=1).astype(np.float64)              # (B,)
    w = np.where(valid, 1.0 / (np.maximum(cnt_t, 1.0) * n_runs[:, None]), 0.0)
    return w.astype(np.float32)


def _plan(audio_lengths):
    """Cut the global valid-frame stream into 8 equal per-core streams.

    Returns per-core segment lists [(row, lo, hi), ...] plus the uniform
    NCHUNK (128-frame chunks per core) and SEGS (max segments per core).
    """
    L = np.minimum(np.asarray(audio_lengths).astype(np.int64), T)
    cum = np.concatenate([[0], np.cumsum(L)])
    F = int(cum[-1])
    cuts = [(F * i) // NCORES for i in range(NCORES + 1)]
    segs = []
    for i in range(NCORES):
        lo_g, hi_g = cuts[i], cuts[i + 1]
        b0 = int(np.searchsorted(cum, lo_g, side="right")) - 1
        s = []
        g = lo_g
        b = b0
        while g < hi_g:
            e = min(int(cum[b + 1]), hi_g)
            s.append((b, g - int(cum[b]), e - int(cum[b])))
            g = e
            b += 1
        segs.append(s)
    SEGS = max(len(s) for s in segs)
    FRC = max(cuts[i + 1] - cuts[i] for i in range(NCORES))
    NCHUNK = -(-FRC // PF)
    return segs, NCHUNK, SEGS


def _build_program(NCHUNK, SEGS):
    import concourse.bacc as bacc
    import concourse.tile as tile
    from concourse import mybir

    f16 = mybir.dt.float16
    f32 = mybir.dt.float32

    nc = bacc.Bacc("TRN2", target_bir_lowering=False, debug=False)
    h = nc.dram_tensor("h", [128, NCHUNK * C], f16, kind="ExternalInput").ap()
    wt = nc.dram_tensor("wt", [128, NCHUNK * SEGS], f16, kind="ExternalInput").ap()
    wv = nc.dram_tensor("wv", [SEGS, C], f32, kind="ExternalInput").ap()
    out = nc.dram_tensor("out", [SEGS, 1], f32, kind="ExternalOutput").ap()

    npieces = -(-NCHUNK // CPT)

    with tile.TileContext(nc) as tc:
        with (
            tc.tile_pool(name="hp", bufs=10) as hp,
            tc.tile_pool(name="const", bufs=1) as cp,
            tc.tile_pool(name="ps", bufs=1, space="PSUM") as pp,
            tc.tile_pool(name="fin", bufs=1) as fp,
        ):
            # Weights ride the fast HWDGE rings ahead of the h stream (the
            # SWDGE path runs ~90 GB/s and would gate the first matmul).
            wtile = cp.tile([128, NCHUNK * SEGS], f16)
            half = (NCHUNK * SEGS) // 2
            nc.sync.dma_start(wtile[:, :half], wt[:, :half])
            nc.scalar.dma_start(wtile[:, half:], wt[:, half:])
            wvt = cp.tile([SEGS, C], f32)
            nc.gpsimd.dma_start(wvt[:], wv)
            logits = cp.tile([SEGS, 1], f32)

            ps = pp.tile([SEGS, C], f32)
            for pc in range(npieces):
                n = min(CPT, NCHUNK - pc * CPT)
                ring = nc.sync if pc % 2 == 0 else nc.scalar
                ht = hp.tile([128, CPT * C], f16, tag="h")
                ring.dma_start(
                    ht[:, : n * C], h[:, pc * CPT * C : (pc * CPT + n) * C]
                )
                for j in range(n):
                    c = pc * CPT + j
                    lw = wtile[:, c * SEGS : (c + 1) * SEGS]
                    nc.tensor.matmul(
                        ps[:, :512], lw, ht[:, j * C : j * C + 512],
                        start=(c == 0), stop=(c == NCHUNK - 1),
                    )
                    nc.tensor.matmul(
                        ps[:, 512:], lw, ht[:, j * C + 512 : (j + 1) * C],
                        start=(c == 0), stop=(c == NCHUNK - 1),
                    )

            sc = fp.tile([SEGS, C], f32)
            nc.vector.scalar_tensor_tensor(
                out=sc[:],
                in0=ps[:],
                scalar=1.0,
                in1=wvt[:],
                op0=mybir.AluOpType.mult,
                op1=mybir.AluOpType.mult,
                accum_out=logits[:],
            )
            nc.sync.dma_start(out, logits[:])

    nc.compile()
    return nc


def _get_program(NCHUNK, SEGS):
    key = (NCHUNK, SEGS)
    if key not in _CACHE:
        _CACHE[key] = _build_program(*key)
    return _CACHE[key]


def _pack(hidden, w, segs, NCHUNK, SEGS):
    """Per-core packed fp16 frame stream (partition-major) and weight matrix."""
    h_maps, wt_maps = [], []
    for s in segs:
        hbuf = np.zeros((NCHUNK * PF, C), dtype=np.float16)
        q = 0
        wvals = []
        sids = []
        for sid, (b, lo, hi) in enumerate(s):
            n = hi - lo
            hbuf[q : q + n] = hidden[b, lo:hi]
            wvals.append(w[b, lo:hi])
            sids.append(np.full(n, sid, dtype=np.int64))
            q += n
        h_maps.append(
            np.ascontiguousarray(
                hbuf.reshape(NCHUNK, PF, C).transpose(1, 0, 2)
            ).reshape(128, NCHUNK * C)
        )
        wvals = np.concatenate(wvals).astype(np.float16)
        sids = np.concatenate(sids)
        pos = np.arange(q)
        wbuf = np.zeros((128, NCHUNK * SEGS), dtype=np.float16)
        wbuf[pos % PF, (pos // PF) * SEGS + sids] = wvals
        wt_maps.append(wbuf)
    return h_maps, wt_maps


def _run(inputs, trace=False):
    from concourse.bass_utils import run_bass_kernel_spmd

    hidden = np.asarray(inputs["hidden_states"], dtype=np.float32)
    W = np.ascontiguousarray(np.asarray(inputs["W"], dtype=np.float32)).reshape(1, C)
    bias = np.asarray(inputs["b"], dtype=np.float32)
    w = _frame_weights(inputs["phoneme_ids"], inputs["audio_lengths"])
    segs, NCHUNK, SEGS = _plan(inputs["audio_lengths"])
    h_maps, wt_maps = _pack(hidden, w, segs, NCHUNK, SEGS)
    wv_rep = np.ascontiguousarray(np.repeat(W, SEGS, axis=0))

    in_maps = [
        {"h": h_maps[i], "wt": wt_maps[i], "wv": wv_rep} for i in range(NCORES)
    ]

    nc = _get_program(NCHUNK, SEGS)
    res = run_bass_kernel_spmd(nc, in_maps, list(range(NCORES)), trace=trace)
    logit = np.zeros((B, 1), dtype=np.float64)
    for i in range(NCORES):
        o = res.results[i]["out"]
        for sid, (b, lo, hi) in enumerate(segs[i]):
            logit[b, 0] += float(o[sid, 0])
    logit = logit.astype(np.float32) + bias[None, :]
    return logit.astype(np.float32), res


def kernel(**inputs):
    return _run(inputs, trace=False)[0]
